# revision 48
# baseline (speedup 1.0000x reference)
"""AttentionHead kernel for 8 Trainium2 NeuronCores.

Problem: x[4,2048,1024] -> Q/K/V projections (qkv_dim=128) -> softmax(Q K^T / sqrt(128)) @ V.

Sharding: core c handles batch b=c//2, query half h=c%2 (1024 queries), with the
full 2048-key sequence for that batch kept local (data-parallel over batch x
query-split; the SxS score matrix stays on-core per the hint). K/V rows are
processed in the order [this core's query half, other half] - softmax and the
attention-weighted sum are permutation-invariant over keys, so each core can
consume the two halves in its own order and no re-indexing is needed.

Per-core pipeline (fp16 compute, fp32 PSUM accumulation everywhere):
 1. x rows stream HBM->SBUF mostly through SWDGE cast-DMAs (inline
    fp32->fp16). x^T [d, s] is produced by PE transposes for the first 8
    s-tiles (so the PE has work immediately) and by XBAR DMA-transposes for
    the last 8 (all copy-DMAs are fenced before any XBAR: mixing the two DMA
    modes makes Tile serialize every transition, ~3us per flip, and
    concurrent XBARs on both HWDGE queues corrupt data).
 2. Projections contract d in 8 128-chunks: W.T @ x^T accumulated in PSUM
    (fp32), ACT copyback fuses the per-partition bias and rounds to fp16,
    giving Q^T/K^T/V^T in [e, s] layout; PE transposes turn V^T into natural
    V [k, e].
 3. Attention runs transposed and is software-pipelined with the projections
    in three phases (columns 0-511 / 512-1023 / 1024-2047) so the ACT exp
    chain starts as early as possible: scores^T[k,q] = K^T-chunk.T @ Q^T for
    two k-chunks into one 2-bank PSUM tile; one ACT exp per pair fuses the
    1/sqrt(128) scale (no max subtraction needed - scores are ~N(0,1) so exp
    is safely bounded); PV accumulates V.T @ expS^T over the 16 k-chunks in
    PSUM while DVE accumulates the softmax denominators in fp16.
 4. Denominators are cross-partition-reduced by PE-transposing the fp16
    accumulators and DVE-reducing along the free axis; the [e,q] output
    accumulator is PE-transposed back to [q,e] with the 1/sum normalization
    applied as a per-partition scalar in the copyback.
"""

import sys

if "/opt/trn_rl_repo" not in sys.path:
    sys.path.insert(0, "/opt/trn_rl_repo")

import numpy as np

P = 128
D = 1024  # d_model
DC = D // P  # 8 contraction chunks
E = 128  # qkv dim
SQ = 1024  # queries per core
SK = 2048  # keys per core
QT = 512  # query column-block width
NQT = SQ // QT  # 2
NKC = SK // P  # 16 key chunks
NST = SK // P  # 16 s-tiles of x
SCALE = 1.0 / float(np.sqrt(E))

_cache: dict = {}

# Set by the first kernel() call; test harnesses can read .exec_time_ns etc.
LAST_RESULT = None


def _build():
    if "nc" in _cache:
        return _cache["nc"]

    import concourse.tile as tile
    from concourse import bacc, mybir
    from concourse.masks import make_identity

    ACTF = mybir.ActivationFunctionType
    f32 = mybir.dt.float32
    f16 = mybir.dt.float16

    nc = bacc.Bacc("TRN2", target_bir_lowering=False, debug=False, num_devices=8)

    xq_d = nc.dram_tensor("xq", [SQ, D], f32, kind="ExternalInput").ap()
    xo_d = nc.dram_tensor("xo", [SQ, D], f32, kind="ExternalInput").ap()
    # weights arrive host-pre-shuffled to [p, t, e] (wq[p,t,e] = Wq[t*128+p, e])
    wq_d = nc.dram_tensor("wq", [P, DC, E], f32, kind="ExternalInput").ap()
    wk_d = nc.dram_tensor("wk", [P, DC, E], f32, kind="ExternalInput").ap()
    wv_d = nc.dram_tensor("wv", [P, DC, E], f32, kind="ExternalInput").ap()
    bq_d = nc.dram_tensor("bq", [E], f32, kind="ExternalInput").ap()
    bk_d = nc.dram_tensor("bk", [E], f32, kind="ExternalInput").ap()
    bv_d = nc.dram_tensor("bv", [E], f32, kind="ExternalInput").ap()
    out_d = nc.dram_tensor("out", [SQ, E], f32, kind="ExternalOutput").ap()

    with tile.TileContext(nc) as tc:
        with (
            tc.tile_pool(name="const", bufs=1) as const,
            tc.tile_pool(name="xload", bufs=16) as xload,
            tc.tile_pool(name="big", bufs=1) as big,
            tc.tile_pool(name="exps", bufs=12) as exps,
            tc.tile_pool(name="misc", bufs=2) as misc,
            tc.tile_pool(name="ptr", bufs=2, space="PSUM") as ptr,
            tc.tile_pool(name="pacc", bufs=2, space="PSUM") as pacc,
            tc.tile_pool(name="po", bufs=2, space="PSUM") as po,
        ):
            # ---- constants first: small loads that unblock the projections ----
            copy_dmas = []
            const_dmas = []
            identf = const.tile([P, P], f32)
            make_identity(nc, identf)
            ident16 = const.tile([P, P], f16)
            nc.vector.tensor_copy(ident16[:], identf[:])
            # ---- x loads (SWDGE cast-DMA fp32->fp16) ----
            x16 = []
            for st in range(NST):
                src = xq_d if st < NST // 2 else xo_d
                row0 = (st % (NST // 2)) * P
                xt = xload.tile([P, D], f16, tag="xin")
                if st < 2:
                    # HWDGE load + DVE cast: lower first-byte latency than the
                    # SWDGE cast-DMA, so the PE starts transposing sooner.
                    xf_ = xload.tile([P, D], f32, tag="xinf")
                    copy_dmas.append(
                        nc.scalar.dma_start(xf_[:], src[row0 : row0 + P, :])
                    )
                    nc.vector.tensor_copy(xt[:], xf_[:])
                else:
                    copy_dmas.append(
                        nc.gpsimd.dma_start(xt[:], src[row0 : row0 + P, :])
                    )
                x16.append(xt)

            w_sb = {}
            for name, wd in (("q", wq_d), ("k", wk_d), ("v", wv_d)):
                wf = const.tile([P, DC, E], f32, name=f"w{name}f")
                const_dmas.append(nc.sync.dma_start(wf[:], wd[:]))
                w = const.tile([P, DC, E], f16, name=f"w{name}")
                nc.vector.tensor_copy(w[:], wf[:])
                w_sb[name] = w
            b_sb = {}
            for name, bd in (("q", bq_d), ("k", bk_d), ("v", bv_d)):
                b = const.tile([P, 1], f32, name=f"b{name}")
                const_dmas.append(nc.sync.dma_start(b[:], bd[:, None]))
                b_sb[name] = b

            # ---- big persistent tiles ----
            xT = big.tile([P, DC, SK], f16)  # x^T: [d_lo, d_chunk, s]
            qT = big.tile([P, SQ], f16)  # Q^T: [e, q]
            kT = big.tile([P, SK], f16)  # K^T: [e, k]
            vT = big.tile([P, SK], f16)  # V^T: [e, k] (staging)
            v_sb = big.tile([P, NKC, E], f16)  # V natural: [k_lo, k_chunk, e]

            # ---- phase 1: transpose x into xT ----
            # First half (this core's query rows) via PE transposes so the PE
            # has work immediately; second half via XBAR DMA-transposes, which
            # must all run after every copy-DMA completes (mixing the two DMA
            # modes makes Tile serialize every transition - HW xbar hazard).
            from concourse.tile import add_dep_helper

            NPE = NST // 2

            def pe_tr(st):
                for half in range(2):
                    ps = ptr.tile([P, 4 * P], f16, tag="tr")
                    for i in range(4):
                        dc = half * 4 + i
                        nc.tensor.transpose(
                            ps[:, i * P : (i + 1) * P],
                            x16[st][:, dc * P : (dc + 1) * P],
                            ident16[:],
                        )
                    nc.vector.tensor_copy(
                        xT[:, half * 4 : (half + 1) * 4, st * P : (st + 1) * P],
                        ps[:].rearrange("p (i s) -> p i s", i=4),
                    )

            for st in range(4):
                pe_tr(st)

            barrier = [copy_dmas[-1], const_dmas[-1]]
            for st in range(6, NST):
                xb = nc.sync.dma_start_transpose(
                    xT[:, :, st * P : (st + 1) * P], x16[st][:]
                )
                for bd_ in barrier:
                    add_dep_helper(xb.ins, bd_.ins, reason="copies before xbars")

            # ---- phase 2+3+4: projections and attention, pipelined ----
            # Work is emitted in dependency-ready order so the exp chain (the
            # ACT-bound critical path) starts as early as possible:
            #  A: s-tiles 0-3 transposed -> Q/K/V cols 0-511 -> attention kc0-3
            #  B: s-tiles 4-7 -> cols 512-1023 -> qt0 kc4-7, qt1 kc0-7
            #  C: s-tiles 8-15 (XBAR) -> cols 1024-2047 -> both qt kc8-15
            def projw(col0, width, w, b, dst):
                psum = pacc.tile([P, 2 * QT], f32, tag="mm")
                for hh in range(max(1, width // QT)):
                    c0 = col0 + hh * QT
                    wd_ = min(QT, width)
                    for dc in range(DC):
                        nc.tensor.matmul(
                            psum[:, hh * QT : hh * QT + wd_],
                            w[:, dc, :],
                            xT[:, dc, c0 : c0 + wd_],
                            start=(dc == 0),
                            stop=(dc == DC - 1),
                        )
                nc.scalar.activation(
                    dst[:, col0 : col0 + width],
                    psum[:, :width],
                    ACTF.Identity,
                    bias=b[:],
                    scale=1.0,
                )

            acc_o = [
                po.tile([P, QT], f32, tag="acc_o", name=f"acc_o{qt}")
                for qt in range(NQT)
            ]
            asum = [
                [
                    big.tile([P, QT], f16, name=f"asum{qt}{h}")
                    for h in range(2)
                ]
                for qt in range(NQT)
            ]

            es_store = {}

            def att_scores(qt, kp):
                q0 = qt * QT
                kc0 = 2 * kp
                ps = pacc.tile([P, 2 * QT], f32, tag="mm")
                for h in range(2):
                    nc.tensor.matmul(
                        ps[:, h * QT : (h + 1) * QT],
                        kT[:, (kc0 + h) * P : (kc0 + h + 1) * P],
                        qT[:, q0 : q0 + QT],
                        start=True,
                        stop=True,
                    )
                es = exps.tile([P, 2 * QT], f16, tag="exps")
                nc.scalar.activation(es[:], ps[:], ACTF.Exp, scale=SCALE)
                es_store[(qt, kp)] = es

            def att_pv(qt, kp):
                kc0 = 2 * kp
                es = es_store.pop((qt, kp))
                for h in range(2):
                    nc.tensor.matmul(
                        acc_o[qt][:],
                        v_sb[:, kc0 + h, :],
                        es[:, h * QT : (h + 1) * QT],
                        start=(kc0 + h == 0),
                        stop=(kc0 + h == NKC - 1),
                    )
                half = kp // 4
                acc = asum[qt][half]
                tmp = misc.tile([P, QT], f16, tag="tmp16")
                nc.vector.tensor_add(out=tmp[:], in0=es[:, :QT], in1=es[:, QT:])
                if kp % 4 == 0:
                    nc.vector.tensor_copy(acc[:], tmp[:])
                else:
                    nc.vector.tensor_add(out=acc[:], in0=acc[:], in1=tmp[:])

            def att_pair(qt, kp):
                att_scores(qt, kp)
                att_pv(qt, kp)

            def vtr(c0, width):
                for g in range(width // (4 * P)):
                    kc0 = c0 // P + 4 * g
                    ps = ptr.tile([P, 4 * P], f16, tag="tr")
                    for i in range(4):
                        nc.tensor.transpose(
                            ps[:, i * P : (i + 1) * P],
                            vT[:, (kc0 + i) * P : (kc0 + i + 1) * P],
                            ident16[:],
                        )
                    nc.vector.tensor_copy(
                        v_sb[:, kc0 : kc0 + 4, :],
                        ps[:].rearrange("p (i s) -> p i s", i=4),
                    )

            sums4h = [
                [
                    big.tile([P, QT // P], f32, name=f"sums4{qt}{h}")
                    for h in range(2)
                ]
                for qt in range(NQT)
            ]

            def sums_half(qt, half):
                ps_a = ptr.tile([P, 4 * P], f16, tag="tr")
                for j in range(QT // P):
                    nc.tensor.transpose(
                        ps_a[:, j * P : (j + 1) * P],
                        asum[qt][half][:, j * P : (j + 1) * P],
                        ident16[:],
                    )
                nc.vector.reduce_sum(
                    sums4h[qt][half][:, :, None],
                    ps_a[:].rearrange("p (j s) -> p j s", j=QT // P),
                    axis=mybir.AxisListType.X,
                )

            # phase A
            projw(0, QT, w_sb["q"], b_sb["q"], qT)
            projw(0, QT, w_sb["k"], b_sb["k"], kT)
            projw(0, QT, w_sb["v"], b_sb["v"], vT)
            vtr(0, QT)
            att_pair(0, 0)
            att_pair(0, 1)
            # phase B
            for st in range(4, 6):
                pe_tr(st)
            projw(QT, QT, w_sb["q"], b_sb["q"], qT)
            projw(QT, QT, w_sb["k"], b_sb["k"], kT)
            projw(QT, QT, w_sb["v"], b_sb["v"], vT)
            vtr(QT, QT)
            att_pair(0, 2)
            att_pair(0, 3)
            for kp in range(4):
                att_pair(1, kp)
            sums_half(0, 0)
            sums_half(1, 0)
            # phase C
            projw(2 * QT, 2 * QT, w_sb["k"], b_sb["k"], kT)
            projw(2 * QT, 2 * QT, w_sb["v"], b_sb["v"], vT)
            vtr(2 * QT, 2 * QT)
            for kp in range(4, 8):
                att_pair(0, kp)

            # ---- tails: normalize and emit ----
            def tail(qt):
                q0 = qt * QT
                sums_half(qt, 1)
                sums4 = misc.tile([P, QT // P], f32, tag="sums4")
                nc.vector.tensor_add(
                    out=sums4[:], in0=sums4h[qt][0][:], in1=sums4h[qt][1][:]
                )
                recip4 = misc.tile([P, QT // P], f32, tag="recip4")
                nc.vector.reciprocal_approx_fast(recip4[:], sums4[:])
                otn = misc.tile([P, QT], f32, tag="otn")
                nc.vector.tensor_copy(otn[:], acc_o[qt][:])
                out_sb = misc.tile([P, QT // P, E], f32, tag="outsb")
                ps_o = ptr.tile([P, 4 * P], f32, tag="tr")
                for j in range(QT // P):
                    nc.tensor.transpose(
                        ps_o[:, j * P : (j + 1) * P],
                        otn[:, j * P : (j + 1) * P],
                        identf[:],
                    )
                for j in range(QT // P):
                    if j % 2 == 0:
                        nc.vector.tensor_scalar_mul(
                            out_sb[:, j, :],
                            ps_o[:, j * P : (j + 1) * P],
                            recip4[:, j : j + 1],
                        )
                    else:
                        nc.scalar.activation(
                            out_sb[:, j, :],
                            ps_o[:, j * P : (j + 1) * P],
                            ACTF.Identity,
                            bias=0.0,
                            scale=recip4[:, j : j + 1],
                        )
                half_q = QT // 2
                for g in range(2):
                    nc.sync.dma_start(
                        out_d[q0 + g * half_q : q0 + (g + 1) * half_q, :].rearrange(
                            "(t p) e -> p t e", p=P
                        ),
                        out_sb[:, g * 2 : (g + 1) * 2, :],
                    )

            tail(0)
            for kp in range(4, 8):
                att_pair(1, kp)
            tail(1)

    nc.compile()
    _cache["nc"] = nc
    return nc


def kernel(x, Wq, bq, Wk, bk, Wv, bv):
    global LAST_RESULT
    nc = _build()
    from concourse import bass_utils

    x = np.asarray(x, dtype=np.float32)
    def _shuf(w):
        w = np.asarray(w, dtype=np.float32).reshape(DC, P, E)
        return np.ascontiguousarray(w.transpose(1, 0, 2))

    Wq, Wk, Wv = _shuf(Wq), _shuf(Wk), _shuf(Wv)
    bq = np.ascontiguousarray(np.asarray(bq, dtype=np.float32))
    bk = np.ascontiguousarray(np.asarray(bk, dtype=np.float32))
    bv = np.ascontiguousarray(np.asarray(bv, dtype=np.float32))
    B, S, _ = x.shape

    in_maps = []
    for c in range(8):
        b, h = c // 2, c % 2
        xq = np.ascontiguousarray(x[b, h * SQ : (h + 1) * SQ])
        xo = np.ascontiguousarray(x[b, (1 - h) * SQ : (2 - h) * SQ])
        in_maps.append(
            {
                "xq": xq,
                "xo": xo,
                "wq": Wq,
                "wk": Wk,
                "wv": Wv,
                "bq": bq,
                "bk": bk,
                "bv": bv,
            }
        )

    res = bass_utils.run_bass_kernel_spmd(nc, in_maps, core_ids=list(range(8)))
    LAST_RESULT = res

    out = np.empty((B, S, E), dtype=np.float32)
    for c in range(8):
        b, h = c // 2, c % 2
        out[b, h * SQ : (h + 1) * SQ] = res.results[c]["out"]
    return out


# revision 49
# speedup vs baseline: 1.0753x; 1.0753x over previous
"""AttentionHead kernel for 8 Trainium2 NeuronCores.

Problem: x[4,2048,1024] -> Q/K/V projections (qkv_dim=128) -> softmax(Q K^T / sqrt(128)) @ V.

Sharding: core c handles batch b=c//2, query half h=c%2 (1024 queries), with the
full 2048-key sequence for that batch kept local (data-parallel over batch x
query-split; the SxS score matrix stays on-core per the hint). K/V rows are
processed in the order [this core's query half, other half] - softmax and the
attention-weighted sum are permutation-invariant over keys, so each core can
consume the two halves in its own order and no re-indexing is needed.

Per-core pipeline (fp16 compute, fp32 PSUM accumulation everywhere):
 1. x rows stream HBM->SBUF mostly through SWDGE cast-DMAs (inline
    fp32->fp16). x^T [d, s] is produced by PE transposes for the first 8
    s-tiles (so the PE has work immediately) and by XBAR DMA-transposes for
    the last 8 (all copy-DMAs are fenced before any XBAR: mixing the two DMA
    modes makes Tile serialize every transition, ~3us per flip, and
    concurrent XBARs on both HWDGE queues corrupt data).
 2. Projections contract d in 8 128-chunks: W.T @ x^T accumulated in PSUM
    (fp32), ACT copyback fuses the per-partition bias and rounds to fp16,
    giving Q^T/K^T/V^T in [e, s] layout; PE transposes turn V^T into natural
    V [k, e].
 3. Attention runs transposed and is software-pipelined with the projections
    in three phases (columns 0-511 / 512-1023 / 1024-2047) so the ACT exp
    chain starts as early as possible: scores^T[k,q] = K^T-chunk.T @ Q^T for
    two k-chunks into one 2-bank PSUM tile; one ACT exp per pair fuses the
    1/sqrt(128) scale (no max subtraction needed - scores are ~N(0,1) so exp
    is safely bounded); PV accumulates V.T @ expS^T over the 16 k-chunks in
    PSUM while DVE accumulates the softmax denominators in fp16.
 4. Denominators are cross-partition-reduced by PE-transposing the fp16
    accumulators and DVE-reducing along the free axis; the [e,q] output
    accumulator is PE-transposed back to [q,e] with the 1/sum normalization
    applied as a per-partition scalar in the copyback.
"""

import sys

if "/opt/trn_rl_repo" not in sys.path:
    sys.path.insert(0, "/opt/trn_rl_repo")

import numpy as np

P = 128
D = 1024  # d_model
DC = D // P  # 8 contraction chunks
E = 128  # qkv dim
SQ = 1024  # queries per core
SK = 2048  # keys per core
QT = 512  # query column-block width
NQT = SQ // QT  # 2
NKC = SK // P  # 16 key chunks
NST = SK // P  # 16 s-tiles of x
SCALE = 1.0 / float(np.sqrt(E))

_cache: dict = {}

# Set by the first kernel() call; test harnesses can read .exec_time_ns etc.
LAST_RESULT = None


def _build():
    if "nc" in _cache:
        return _cache["nc"]

    import concourse.tile as tile
    from concourse import bacc, mybir
    from concourse.masks import make_identity

    ACTF = mybir.ActivationFunctionType
    f32 = mybir.dt.float32
    f16 = mybir.dt.float16

    nc = bacc.Bacc("TRN2", target_bir_lowering=False, debug=False, num_devices=8)

    xq_d = nc.dram_tensor("xq", [SQ, D], f32, kind="ExternalInput").ap()
    xo_d = nc.dram_tensor("xo", [SQ, D], f32, kind="ExternalInput").ap()
    # weights arrive host-pre-shuffled to [p, t, e] (wq[p,t,e] = Wq[t*128+p, e])
    wq_d = nc.dram_tensor("wq", [P, DC, E], f32, kind="ExternalInput").ap()
    wk_d = nc.dram_tensor("wk", [P, DC, E], f32, kind="ExternalInput").ap()
    wv_d = nc.dram_tensor("wv", [P, DC, E], f32, kind="ExternalInput").ap()
    bq_d = nc.dram_tensor("bq", [E], f32, kind="ExternalInput").ap()
    bk_d = nc.dram_tensor("bk", [E], f32, kind="ExternalInput").ap()
    bv_d = nc.dram_tensor("bv", [E], f32, kind="ExternalInput").ap()
    out_d = nc.dram_tensor("out", [SQ, E], f32, kind="ExternalOutput").ap()

    with tile.TileContext(nc) as tc:
        with (
            tc.tile_pool(name="const", bufs=1) as const,
            tc.tile_pool(name="xload", bufs=16) as xload,
            tc.tile_pool(name="big", bufs=1) as big,
            tc.tile_pool(name="exps", bufs=12) as exps,
            tc.tile_pool(name="misc", bufs=2) as misc,
            tc.tile_pool(name="ptr", bufs=2, space="PSUM") as ptr,
            tc.tile_pool(name="pacc", bufs=2, space="PSUM") as pacc,
            tc.tile_pool(name="po", bufs=2, space="PSUM") as po,
        ):
            # ---- constants first: small loads that unblock the projections ----
            copy_dmas = []
            const_dmas = []
            identf = const.tile([P, P], f32)
            make_identity(nc, identf)
            ident16 = const.tile([P, P], f16)
            nc.vector.tensor_copy(ident16[:], identf[:])
            # ---- x loads (SWDGE cast-DMA fp32->fp16) ----
            x16 = []
            for st in range(NST):
                src = xq_d if st < NST // 2 else xo_d
                row0 = (st % (NST // 2)) * P
                xt = xload.tile([P, D], f16, tag="xin")
                if st < 2:
                    # HWDGE load + DVE cast: lower first-byte latency than the
                    # SWDGE cast-DMA, so the PE starts transposing sooner.
                    xf_ = xload.tile([P, D], f32, tag="xinf")
                    copy_dmas.append(
                        nc.scalar.dma_start(xf_[:], src[row0 : row0 + P, :])
                    )
                    nc.vector.tensor_copy(xt[:], xf_[:])
                else:
                    copy_dmas.append(
                        nc.gpsimd.dma_start(xt[:], src[row0 : row0 + P, :])
                    )
                x16.append(xt)

            w_sb = {}
            for name, wd in (("q", wq_d), ("k", wk_d), ("v", wv_d)):
                wf = const.tile([P, DC, E], f32, name=f"w{name}f")
                const_dmas.append(nc.sync.dma_start(wf[:], wd[:]))
                w = const.tile([P, DC, E], f16, name=f"w{name}")
                nc.vector.tensor_copy(w[:], wf[:])
                w_sb[name] = w
            b_sb = {}
            for name, bd in (("q", bq_d), ("k", bk_d), ("v", bv_d)):
                b = const.tile([P, 1], f32, name=f"b{name}")
                const_dmas.append(nc.sync.dma_start(b[:], bd[:, None]))
                b_sb[name] = b

            # ---- big persistent tiles ----
            xT = big.tile([P, DC, SK], f16)  # x^T: [d_lo, d_chunk, s]
            qT = big.tile([P, SQ], f16)  # Q^T: [e, q]
            kT = big.tile([P, SK], f16)  # K^T: [e, k]
            vT = big.tile([P, SK], f16)  # V^T: [e, k] (staging)
            v_sb = big.tile([P, NKC, E], f16)  # V natural: [k_lo, k_chunk, e]

            # ---- phase 1: transpose x into xT ----
            # First half (this core's query rows) via PE transposes so the PE
            # has work immediately; second half via XBAR DMA-transposes, which
            # must all run after every copy-DMA completes (mixing the two DMA
            # modes makes Tile serialize every transition - HW xbar hazard).
            from concourse.tile import add_dep_helper

            NPE = NST // 2

            def pe_tr(st):
                for half in range(2):
                    ps = ptr.tile([P, 4 * P], f16, tag="tr")
                    for i in range(4):
                        dc = half * 4 + i
                        nc.tensor.transpose(
                            ps[:, i * P : (i + 1) * P],
                            x16[st][:, dc * P : (dc + 1) * P],
                            ident16[:],
                        )
                    nc.vector.tensor_copy(
                        xT[:, half * 4 : (half + 1) * 4, st * P : (st + 1) * P],
                        ps[:].rearrange("p (i s) -> p i s", i=4),
                    )

            for st in range(4):
                pe_tr(st)

            barrier = [copy_dmas[-1], const_dmas[-1]]
            for st in range(NPE, NST):
                xb = nc.sync.dma_start_transpose(
                    xT[:, :, st * P : (st + 1) * P], x16[st][:]
                )
                for bd_ in barrier:
                    add_dep_helper(xb.ins, bd_.ins, reason="copies before xbars")

            # ---- phase 2+3+4: projections and attention, pipelined ----
            # Work is emitted in dependency-ready order so the exp chain (the
            # ACT-bound critical path) starts as early as possible:
            #  A: s-tiles 0-3 transposed -> Q/K/V cols 0-511 -> attention kc0-3
            #  B: s-tiles 4-7 -> cols 512-1023 -> qt0 kc4-7, qt1 kc0-7
            #  C: s-tiles 8-15 (XBAR) -> cols 1024-2047 -> both qt kc8-15
            def projw(col0, width, w, b, dst):
                psum = pacc.tile([P, 2 * QT], f32, tag="mm")
                for hh in range(max(1, width // QT)):
                    c0 = col0 + hh * QT
                    wd_ = min(QT, width)
                    for dc in range(DC):
                        nc.tensor.matmul(
                            psum[:, hh * QT : hh * QT + wd_],
                            w[:, dc, :],
                            xT[:, dc, c0 : c0 + wd_],
                            start=(dc == 0),
                            stop=(dc == DC - 1),
                        )
                nc.scalar.activation(
                    dst[:, col0 : col0 + width],
                    psum[:, :width],
                    ACTF.Identity,
                    bias=b[:],
                    scale=1.0,
                )

            acc_o = [
                po.tile([P, QT], f32, tag="acc_o", name=f"acc_o{qt}")
                for qt in range(NQT)
            ]
            asum = [
                [
                    big.tile([P, QT], f16, name=f"asum{qt}{h}")
                    for h in range(2)
                ]
                for qt in range(NQT)
            ]

            es_store = {}

            def att_scores(qt, kp):
                q0 = qt * QT
                kc0 = 2 * kp
                ps = pacc.tile([P, 2 * QT], f32, tag="mm")
                for h in range(2):
                    nc.tensor.matmul(
                        ps[:, h * QT : (h + 1) * QT],
                        kT[:, (kc0 + h) * P : (kc0 + h + 1) * P],
                        qT[:, q0 : q0 + QT],
                        start=True,
                        stop=True,
                    )
                es = exps.tile([P, 2 * QT], f16, tag="exps")
                nc.scalar.activation(es[:], ps[:], ACTF.Exp, scale=SCALE)
                es_store[(qt, kp)] = es

            def att_pv(qt, kp):
                kc0 = 2 * kp
                es = es_store.pop((qt, kp))
                for h in range(2):
                    nc.tensor.matmul(
                        acc_o[qt][:],
                        v_sb[:, kc0 + h, :],
                        es[:, h * QT : (h + 1) * QT],
                        start=(kc0 + h == 0),
                        stop=(kc0 + h == NKC - 1),
                    )
                half = kp // 4
                acc = asum[qt][half]
                tmp = misc.tile([P, QT], f16, tag="tmp16")
                nc.vector.tensor_add(out=tmp[:], in0=es[:, :QT], in1=es[:, QT:])
                if kp % 4 == 0:
                    nc.vector.tensor_copy(acc[:], tmp[:])
                else:
                    nc.vector.tensor_add(out=acc[:], in0=acc[:], in1=tmp[:])

            def att_pair(qt, kp):
                att_scores(qt, kp)
                att_pv(qt, kp)

            def vtr(c0, width):
                for g in range(width // (4 * P)):
                    kc0 = c0 // P + 4 * g
                    ps = ptr.tile([P, 4 * P], f16, tag="tr")
                    for i in range(4):
                        nc.tensor.transpose(
                            ps[:, i * P : (i + 1) * P],
                            vT[:, (kc0 + i) * P : (kc0 + i + 1) * P],
                            ident16[:],
                        )
                    nc.vector.tensor_copy(
                        v_sb[:, kc0 : kc0 + 4, :],
                        ps[:].rearrange("p (i s) -> p i s", i=4),
                    )

            sums4h = [
                [
                    big.tile([P, QT // P], f32, name=f"sums4{qt}{h}")
                    for h in range(2)
                ]
                for qt in range(NQT)
            ]

            def sums_half(qt, half):
                ps_a = ptr.tile([P, 4 * P], f16, tag="tr")
                for j in range(QT // P):
                    nc.tensor.transpose(
                        ps_a[:, j * P : (j + 1) * P],
                        asum[qt][half][:, j * P : (j + 1) * P],
                        ident16[:],
                    )
                nc.vector.reduce_sum(
                    sums4h[qt][half][:, :, None],
                    ps_a[:].rearrange("p (j s) -> p j s", j=QT // P),
                    axis=mybir.AxisListType.X,
                )

            # phase A
            projw(0, QT, w_sb["q"], b_sb["q"], qT)
            projw(0, QT, w_sb["k"], b_sb["k"], kT)
            projw(0, QT, w_sb["v"], b_sb["v"], vT)
            vtr(0, QT)
            att_pair(0, 0)
            att_pair(0, 1)
            # phase B
            for st in range(4, 8):
                pe_tr(st)
            projw(QT, QT, w_sb["q"], b_sb["q"], qT)
            projw(QT, QT, w_sb["k"], b_sb["k"], kT)
            projw(QT, QT, w_sb["v"], b_sb["v"], vT)
            vtr(QT, QT)
            att_pair(0, 2)
            att_pair(0, 3)
            for kp in range(4):
                att_pair(1, kp)
            sums_half(0, 0)
            sums_half(1, 0)
            # phase C
            projw(2 * QT, 2 * QT, w_sb["k"], b_sb["k"], kT)
            projw(2 * QT, 2 * QT, w_sb["v"], b_sb["v"], vT)
            vtr(2 * QT, 2 * QT)
            for kp in range(4, 8):
                att_pair(0, kp)

            # ---- tails: normalize and emit ----
            def tail(qt):
                q0 = qt * QT
                sums_half(qt, 1)
                sums4 = misc.tile([P, QT // P], f32, tag="sums4")
                nc.vector.tensor_add(
                    out=sums4[:], in0=sums4h[qt][0][:], in1=sums4h[qt][1][:]
                )
                recip4 = misc.tile([P, QT // P], f32, tag="recip4")
                nc.vector.reciprocal_approx_fast(recip4[:], sums4[:])
                otn = misc.tile([P, QT], f32, tag="otn")
                nc.vector.tensor_copy(otn[:], acc_o[qt][:])
                out_sb = misc.tile([P, QT // P, E], f32, tag="outsb")
                ps_o = ptr.tile([P, 4 * P], f32, tag="tr")
                for j in range(QT // P):
                    nc.tensor.transpose(
                        ps_o[:, j * P : (j + 1) * P],
                        otn[:, j * P : (j + 1) * P],
                        identf[:],
                    )
                for j in range(QT // P):
                    if j % 2 == 0:
                        nc.vector.tensor_scalar_mul(
                            out_sb[:, j, :],
                            ps_o[:, j * P : (j + 1) * P],
                            recip4[:, j : j + 1],
                        )
                    else:
                        nc.scalar.activation(
                            out_sb[:, j, :],
                            ps_o[:, j * P : (j + 1) * P],
                            ACTF.Identity,
                            bias=0.0,
                            scale=recip4[:, j : j + 1],
                        )
                half_q = QT // 2
                for g in range(2):
                    nc.sync.dma_start(
                        out_d[q0 + g * half_q : q0 + (g + 1) * half_q, :].rearrange(
                            "(t p) e -> p t e", p=P
                        ),
                        out_sb[:, g * 2 : (g + 1) * 2, :],
                    )

            tail(0)
            for kp in range(4, 8):
                att_pair(1, kp)
            tail(1)

    nc.compile()
    _cache["nc"] = nc
    return nc


def kernel(x, Wq, bq, Wk, bk, Wv, bv):
    global LAST_RESULT
    nc = _build()
    from concourse import bass_utils

    x = np.asarray(x, dtype=np.float32)
    def _shuf(w):
        w = np.asarray(w, dtype=np.float32).reshape(DC, P, E)
        return np.ascontiguousarray(w.transpose(1, 0, 2))

    Wq, Wk, Wv = _shuf(Wq), _shuf(Wk), _shuf(Wv)
    bq = np.ascontiguousarray(np.asarray(bq, dtype=np.float32))
    bk = np.ascontiguousarray(np.asarray(bk, dtype=np.float32))
    bv = np.ascontiguousarray(np.asarray(bv, dtype=np.float32))
    B, S, _ = x.shape

    in_maps = []
    for c in range(8):
        b, h = c // 2, c % 2
        xq = np.ascontiguousarray(x[b, h * SQ : (h + 1) * SQ])
        xo = np.ascontiguousarray(x[b, (1 - h) * SQ : (2 - h) * SQ])
        in_maps.append(
            {
                "xq": xq,
                "xo": xo,
                "wq": Wq,
                "wk": Wk,
                "wv": Wv,
                "bq": bq,
                "bk": bk,
                "bv": bv,
            }
        )

    res = bass_utils.run_bass_kernel_spmd(nc, in_maps, core_ids=list(range(8)))
    LAST_RESULT = res

    out = np.empty((B, S, E), dtype=np.float32)
    for c in range(8):
        b, h = c // 2, c % 2
        out[b, h * SQ : (h + 1) * SQ] = res.results[c]["out"]
    return out
